# revision 1
# baseline (speedup 1.0000x reference)
"""Trainium2 Bass kernel for nn_KeypointLoss (S=3, B=8, K=11, C=23, H=W=256).

Data-parallel over batch B=8 across 8 NeuronCores: core b computes the three
losses (heatmap / label / mask) for batch element b; host assembles [B,S].

Per-core device algorithm (all loss math on device), per stack s:
  heat : one batched DVE mul (mask broadcast over K), one DVE sub, one ACT
         Square with accum -> acc col s
  label: per-plane argmax = DVE row-max + PE transpose + one-hot; winning gt
         row re-fetched via one indirect DMA to recover the column; the 7
         label-channel values gathered with one width-1 indirect DMA; BCE on
         [11,7]
  mask : BCE via ACT Ln(+accum) and DVE fused mul-reduce
  final: two small matmuls reduce partition partials -> out[1,9]
"""

import numpy as np

S = 3
B = 8
K = 11
C = 23
P = 128
F = 512  # 256*256 = 128*512 plane layout
NACC = 12  # 3 heat + 3 ln1mp + 3 g*dd + 3 label cols

_CACHE = {}


def _build_nc():
    import concourse.bass as bass
    import concourse.bacc as bacc
    import concourse.mybir as mybir
    import concourse.tile as tile

    dt = mybir.dt
    f32, i32 = dt.float32, dt.int32
    Alu = mybir.AluOpType
    Act = mybir.ActivationFunctionType
    AX = mybir.AxisListType.X

    # Bacc (not raw Bass): its compile pipeline splits multi-wait sync into
    # event semaphores (TRN2 allows one wait per instruction)
    nc = bacc.Bacc("TRN2", target_bir_lowering=False, debug=False)
    cp = nc.declare_dram_parameter("cp", [S, C, P, F], f32, isOutput=False)
    hm = nc.declare_dram_parameter("hm", [S, K, P, F], f32, isOutput=False)
    mk = nc.declare_dram_parameter("mk", [S, P, F], f32, isOutput=False)
    lab = nc.declare_dram_parameter("lab", [K, 7], f32, isOutput=False)
    wmp = nc.declare_dram_parameter("wm", [NACC, 9], f32, isOutput=False)
    idp = nc.declare_dram_parameter("ident", [128, 128], f32, isOutput=False)
    iop = nc.declare_dram_parameter("iotap", [K, 128], f32, isOutput=False)
    iof = nc.declare_dram_parameter("iotaf", [K, F], f32, isOutput=False)
    k1p = nc.declare_dram_parameter("k128", [K, 1], f32, isOutput=False)
    cvp = nc.declare_dram_parameter("cvec", [K, 8], f32, isOutput=False)
    out = nc.declare_dram_parameter("out", [1, 16], f32, isOutput=True)

    hm_flat = hm[:].rearrange("s k p f -> (s k p) f")     # 512-wide rows
    cp_pix = cp[:].rearrange("s c p (f one) -> (s c p f) one", one=1)  # width-1 pixel rows

    with tile.TileContext(nc) as tc:
        with (
            tc.tile_pool(name="const", bufs=1) as cst,
            tc.tile_pool(name="accp", bufs=1) as accp,
            tc.tile_pool(name="big", bufs=3) as big,
            tc.tile_pool(name="sm", bufs=2) as sm,
            tc.tile_pool(name="ps", bufs=2, space="PSUM") as ps,
        ):
            # ---------------- constants (host-provided) ----------------
            ident_d = cst.tile([128, 128], f32)
            nc.sync.dma_start(out=ident_d[:], in_=idp[:])
            ident = cst.tile([128, 128], f32)
            nc.vector.tensor_copy(ident[:], ident_d[:])
            iotaPf = cst.tile([K, 128], f32)
            nc.sync.dma_start(out=iotaPf[:], in_=iop[:])
            iotaFf = cst.tile([K, F], f32)
            nc.sync.dma_start(out=iotaFf[:], in_=iof[:])
            k128f = cst.tile([K, 1], f32)
            nc.sync.dma_start(out=k128f[:], in_=k1p[:])
            cvec = cst.tile([K, 8], f32)
            nc.sync.dma_start(out=cvec[:], in_=cvp[:])
            ones = cst.tile([128, 1], f32)
            nc.vector.memset(ones[:], 1.0)
            Wm_d = cst.tile([NACC, 9], f32)
            nc.sync.dma_start(out=Wm_d[:], in_=wmp[:])
            Wm = cst.tile([NACC, 9], f32)
            nc.vector.tensor_copy(Wm[:], Wm_d[:])
            labsb = cst.tile([K, 7], f32)
            nc.sync.dma_start(out=labsb[:], in_=lab[:])

            acc = accp.tile([128, NACC], f32)
            nc.vector.memset(acc[:], 0.0)

            # ---------------- per-stack main loop ----------------
            for s in range(S):
                pred = big.tile([P, K, F], f32, tag="pred")
                gt = big.tile([P, K, F], f32, tag="gt")
                mask = big.tile([P, F], f32, tag="mask")
                mpred = big.tile([P, F], f32, tag="mpred")
                nc.sync.dma_start(out=gt[:], in_=hm[s].rearrange("k p f -> p k f"))
                nc.sync.dma_start(out=pred[:], in_=cp[s, K:2 * K].rearrange("k p f -> p k f"))
                nc.sync.dma_start(out=mask[:], in_=mk[s])
                nc.sync.dma_start(out=mpred[:], in_=cp[s, 2 * K])

                # ---- heatmap loss: sum_{k,pix} (pred*mask - gt)^2, batched
                mask_b = mask[:].rearrange("p (a f) -> p a f", a=1).to_broadcast([P, K, F])
                nc.vector.tensor_tensor(out=pred[:], in0=pred[:], in1=mask_b, op=Alu.mult)
                nc.vector.tensor_tensor(out=pred[:], in0=pred[:], in1=gt[:], op=Alu.subtract)
                nc.scalar.activation(out=pred[:], in_=pred[:], func=Act.Square,
                                     accum_out=acc[:, s:s + 1])

                # ---- mask loss: BCE(mpred, mask) summed
                ln1_m = big.tile([P, F], f32, tag="ln1m")
                lnp_m = big.tile([P, F], f32, tag="lnpm")
                nc.scalar.activation(out=ln1_m[:], in_=mpred[:], func=Act.Ln,
                                     bias=1.0, scale=-1.0,
                                     accum_out=acc[:, 3 + s:4 + s])
                nc.scalar.activation(out=lnp_m[:], in_=mpred[:], func=Act.Ln)
                nc.gpsimd.tensor_tensor(out=lnp_m[:], in0=lnp_m[:], in1=ln1_m[:],
                                        op=Alu.subtract)
                nc.vector.scalar_tensor_tensor(out=lnp_m[:], in0=lnp_m[:],
                                               scalar=0.0, in1=mask[:],
                                               op0=Alu.bypass, op1=Alu.mult,
                                               accum_out=acc[:, 6 + s:7 + s])

                # ---- label loss: per-plane argmax + gathers + BCE
                rowmax = sm.tile([P, K], f32, tag="rowmax")
                nc.vector.tensor_reduce(out=rowmax[:], in_=gt[:], axis=AX, op=Alu.max)
                pt = ps.tile([K, 128], f32, tag="pt")
                nc.tensor.transpose(out=pt[:], in_=rowmax[:], identity=ident[:])
                rowmaxT = sm.tile([K, 128], f32, tag="rmT")
                nc.vector.tensor_copy(rowmaxT[:], pt[:])
                Mx = sm.tile([K, 1], f32, tag="Mx")
                nc.vector.tensor_reduce(out=Mx[:], in_=rowmaxT[:], axis=AX, op=Alu.max)
                onehotT = sm.tile([K, 128], f32, tag="oh")
                nc.vector.tensor_scalar(out=onehotT[:], in0=rowmaxT[:],
                                        scalar1=Mx[:, 0:1], scalar2=None,
                                        op0=Alu.is_equal)
                scrT = sm.tile([K, 128], f32, tag="scrT")
                pstarf = sm.tile([K, 1], f32, tag="pstar")
                nc.vector.scalar_tensor_tensor(out=scrT[:], in0=onehotT[:],
                                               scalar=0.0, in1=iotaPf[:],
                                               op0=Alu.bypass, op1=Alu.mult,
                                               accum_out=pstarf[:])
                # winning gt row (row index = s*1408 + k*128 + p*)
                idxg_f = sm.tile([K, 1], f32, tag="idxgf")
                nc.vector.scalar_tensor_tensor(out=idxg_f[:], in0=pstarf[:],
                                               scalar=float(s * K * 128), in1=k128f[:],
                                               op0=Alu.add, op1=Alu.add)
                idxg_i = sm.tile([K, 1], i32, tag="idxgi")
                nc.vector.tensor_copy(idxg_i[:], idxg_f[:])
                grow = sm.tile([K, F], f32, tag="grow")
                nc.gpsimd.indirect_dma_start(
                    out=grow[:], out_offset=None, in_=hm_flat,
                    in_offset=bass.IndirectOffsetOnAxis(ap=idxg_i[:, 0:1], axis=0))
                wsel = sm.tile([K, F], f32, tag="wsel")
                nc.vector.tensor_scalar(out=wsel[:], in0=grow[:], scalar1=Mx[:, 0:1],
                                        scalar2=None, op0=Alu.is_equal)
                valid = sm.tile([K, 1], f32, tag="valid")
                nc.vector.tensor_scalar(out=valid[:], in0=Mx[:], scalar1=1.0,
                                        scalar2=None, op0=Alu.is_equal)
                # f* (column of max within the row), then flat pixel index
                scrF = sm.tile([K, F], f32, tag="scrF")
                fstar = sm.tile([K, 1], f32, tag="fstar")
                nc.vector.scalar_tensor_tensor(out=scrF[:], in0=wsel[:],
                                               scalar=0.0, in1=iotaFf[:],
                                               op0=Alu.bypass, op1=Alu.mult,
                                               accum_out=fstar[:])
                fidx = sm.tile([K, 1], f32, tag="fidx")
                nc.vector.scalar_tensor_tensor(out=fidx[:], in0=pstarf[:],
                                               scalar=512.0, in1=fstar[:],
                                               op0=Alu.mult, op1=Alu.add)
                # 8 flat element indices per k: (s*C + c)*65536 + p**512 + f*
                idx8f = sm.tile([K, 8], f32, tag="idx8f")
                nc.vector.scalar_tensor_tensor(
                    out=idx8f[:], in0=fidx[:, 0:1].to_broadcast([K, 8]),
                    scalar=float(s * C * 65536), in1=cvec[:],
                    op0=Alu.add, op1=Alu.add)
                idx8i = sm.tile([K, 8], i32, tag="idx8i")
                nc.vector.tensor_copy(idx8i[:], idx8f[:])
                G8 = sm.tile([K, 8], f32, tag="G8")
                for c in range(7):
                    nc.gpsimd.indirect_dma_start(
                        out=G8[:, c:c + 1], out_offset=None, in_=cp_pix,
                        in_offset=bass.IndirectOffsetOnAxis(ap=idx8i[:, c:c + 1],
                                                            axis=0))
                # BCE over gathered [K,7]
                G = G8[:, 0:7]
                lnp = sm.tile([K, 7], f32, tag="lnp")
                ln1 = sm.tile([K, 7], f32, tag="ln1")
                l1s = sm.tile([K, 1], f32, tag="l1s")
                nc.scalar.activation(out=ln1[:], in_=G, func=Act.Ln,
                                     bias=1.0, scale=-1.0, accum_out=l1s[:])
                nc.scalar.activation(out=lnp[:], in_=G, func=Act.Ln)
                dd = sm.tile([K, 7], f32, tag="dd")
                nc.vector.tensor_tensor(out=dd[:], in0=lnp[:], in1=ln1[:], op=Alu.subtract)
                scr7 = sm.tile([K, 7], f32, tag="scr7")
                wsum = sm.tile([K, 1], f32, tag="wsum")
                nc.vector.tensor_tensor(out=scr7[:], in0=dd[:], in1=labsb[:],
                                        op=Alu.mult)
                nc.vector.tensor_reduce(out=wsum[:], in_=scr7[:], axis=AX, op=Alu.add)
                tsum = sm.tile([K, 1], f32, tag="tsum")
                nc.vector.tensor_tensor(out=tsum[:], in0=wsum[:], in1=l1s[:], op=Alu.add)
                nc.vector.tensor_tensor(out=acc[0:K, 9 + s:10 + s], in0=tsum[:],
                                        in1=valid[:], op=Alu.mult)

            # ---------------- final reduction ----------------
            # stage acc through DVE so the matmul sees few producers
            acc2 = accp.tile([128, NACC], f32)
            nc.vector.tensor_copy(acc2[:], acc[:])
            ps1 = ps.tile([NACC, 1], f32, tag="ps1")
            nc.tensor.matmul(out=ps1[:], lhsT=acc2[:], rhs=ones[:], start=True, stop=True)
            s1 = sm.tile([NACC, 1], f32, tag="s1")
            nc.vector.tensor_copy(s1[:], ps1[:])
            ps2 = ps.tile([1, 9], f32, tag="ps2")
            nc.tensor.matmul(out=ps2[:], lhsT=s1[:], rhs=Wm[:], start=True, stop=True)
            res = sm.tile([1, 16], f32, tag="res")
            nc.vector.memset(res[:], 0.0)
            nc.vector.tensor_copy(res[0:1, 0:9], ps2[:])
            nc.sync.dma_start(out=out[:], in_=res[:])

    nc.finalize()
    return nc


def get_nc():
    if "nc" not in _CACHE:
        _CACHE["nc"] = _build_nc()
    return _CACHE["nc"]


def _make_wm():
    wm = np.zeros((NACC, 9), dtype=np.float32)
    for s in range(S):
        wm[s, s] = 1.0 / 11.0                # heat: accum is sum over K,pix
        wm[3 + s, 3 + s] = -1.0 / 65536.0    # mask: -(A+B)/HW
        wm[6 + s, 3 + s] = -1.0 / 65536.0
        wm[9 + s, 6 + s] = -1.0 / 77.0       # label: -sum/(7*11)
    return wm


def make_in_maps(combined_preds, heatmaps, labels, masks):
    cpn = np.asarray(combined_preds, dtype=np.float32)
    hmn = np.asarray(heatmaps, dtype=np.float32)
    lbn = np.asarray(labels, dtype=np.float32)
    mkn = np.asarray(masks, dtype=np.float32)
    wm = _make_wm()
    ident = np.eye(128, dtype=np.float32)
    iotap = np.broadcast_to(np.arange(128, dtype=np.float32), (K, 128)).copy()
    iotaf = np.broadcast_to(np.arange(F, dtype=np.float32), (K, F)).copy()
    k128 = (np.arange(K, dtype=np.float32) * 128.0).reshape(K, 1)
    cvec = np.broadcast_to(np.arange(8, dtype=np.float32) * 65536.0, (K, 8)).copy()
    in_maps = []
    for b in range(B):
        in_maps.append({
            "cp": np.ascontiguousarray(cpn[:, b]).reshape(S, C, P, F),
            "hm": np.ascontiguousarray(hmn[:, b]).reshape(S, K, P, F),
            "mk": np.ascontiguousarray(mkn[:, b, 0]).reshape(S, P, F),
            "lab": np.ascontiguousarray(lbn[b]),
            "wm": wm,
            "ident": ident,
            "iotap": iotap,
            "iotaf": iotaf,
            "k128": k128,
            "cvec": cvec,
        })
    return in_maps


def run_spmd(in_maps, trace=False, **kw):
    from concourse.bass_utils import run_bass_kernel_spmd
    return run_bass_kernel_spmd(get_nc(), in_maps, core_ids=list(range(B)),
                                trace=trace, **kw)


def kernel(combined_preds, heatmaps, labels, masks):
    res = run_spmd(make_in_maps(combined_preds, heatmaps, labels, masks)).results
    heat = np.stack([res[b]["out"][0, 0:3] for b in range(B)]).astype(np.float32)
    mask_l = np.stack([res[b]["out"][0, 3:6] for b in range(B)]).astype(np.float32)
    label = np.stack([res[b]["out"][0, 6:9] for b in range(B)]).astype(np.float32)
    return (heat, label, mask_l)

